# revision 1
# baseline (speedup 1.0000x reference)
"""Trainium2 Bass kernel for nn_Attention_73718818669284.

Reference computation (per batch b of 2, C=128 channels, N=4096 spatial):
    q = Wq x, k = Wk x, v = Wv x           (1x1 conv == channel matmul)
    w = softmax(q^T k, axis=-1)            ([N, N] attention)
    h = Wo (v w^T)
    y = x + h
    out = SiLU(GroupNorm8(y) * gamma + beta)

Sharding: 8 cores = 2 batches x 4 column-slices of N (1024 each).
Each core computes its slice of the attention output; GroupNorm statistics
are combined with ONE 8-rank AllReduce on a batch-masked [128, 4] payload
(each core contributes its stats in its batch's column pair and selects
its half post-reduce). Two concurrent 4-rank group collectives serialize
on the CC machinery (~15us extra for the second group); the single 8-rank
op avoids that. A warm-up collective at kernel entry wakes the CC cores
and absorbs cross-core start stagger in parallel with the prologue DMAs.

Per-core algorithm (transposed-score layout -> no PE transposes of P):
    M   = Wq^T Wk                     (one 128x128 matmul, fp16)
    R   = M^T X_s                     ([128, 1024] fp16, folds q-projection)
    S^T chunk j = X[:,128j:]^T R      ([128m, 1024n]; scores, fp16 inputs)
    P^T = exp(S^T + shift)            (bf16; shift cancels in softmax)
    rowsum = sum_m P^T[m, n]          (DVE bf16 dual accumulators)
    h_un = V P = sum_j VT_j^T PT_j    (VT_j = X_j^T Wv^T directly, bf16)
    h = h_un * (1/rowsum)             (1/r = exp(-ln r) on ACT, set 6)
    y = Wo h + x_s ; stats exchange; GroupNorm; SiLU.

Matmul dtypes: the score path (X, M, R, Wo/h) runs in fp16 (10-bit
mantissa keeps score errors ~1e-3; bf16 scores measure 3e-2 rel err),
the P-side (exp output, V, rowsums) in bf16 (fp16 would overflow at
e^41). Both stream 1 cycle/row with fast weight load; the fp32 moving
path measures ~2x slower. The ACT table set 6 (exp+ln+square) is pinned
at kernel entry; the only switch (silu) hides under the stats exchange.
GroupNorm rstd is exp(-0.5 ln(var+eps)) on ACT (set 6 resident).
"""

import numpy as np

import concourse.bass as bass
import concourse.tile as tile
from concourse import bacc, mybir
from concourse.bass_utils import run_bass_kernel_spmd

F32 = mybir.dt.float32
F16 = mybir.dt.float16
BF16 = mybir.dt.bfloat16
NPBF16 = mybir.dt.np(mybir.dt.bfloat16)
AF = mybir.ActivationFunctionType
ALU = mybir.AluOpType
AX = mybir.AxisListType

P = 128          # channels / partitions
N = 4096         # spatial size (16*16*16)
NS = 1024        # per-core slice of N
NB = N // P      # 32 m-chunks
NCORES = 8
NGROUPS = 8
EPS = 1e-5
CNT = (P // NGROUPS) * N   # elements per group per batch = 16 * 4096
NPB = 5 * P                # fp16 params width (wq|wk|wvT|woT|ident)
NPF = NGROUPS + 4          # fp32 params width (gsel/CNT | gamma | beta | bsel0 | bsel1)
ACT_SET_MAIN = 6           # natural_log_exp_and_others: exp + ln + square


def _load_act_set(nc, set_id):
    return nc.scalar.add_instruction(
        mybir.InstLoadActFuncSet(
            name=nc.get_next_instruction_name(),
            ins=[], outs=[],
            act_func_set_id=set_id,
        )
    )


def _build_nc():
    nc = bacc.Bacc("TRN2", target_bir_lowering=False, debug=False,
                   num_devices=NCORES)

    xb = nc.declare_dram_parameter("xb", [P, N], F16, isOutput=False)
    xs16 = nc.declare_dram_parameter("xs16", [P, NS], F16, isOutput=False)
    pb = nc.declare_dram_parameter("pb", [P, NPB], F16, isOutput=False)
    pf = nc.declare_dram_parameter("pf", [P, NPF], F32, isOutput=False)
    gselT = nc.declare_dram_parameter("gselT", [NGROUPS, P], F32,
                                      isOutput=False)
    out = nc.declare_dram_parameter("out", [P, NS], F32, isOutput=True)

    with tile.TileContext(nc) as tc:
        _emit(nc, tc, xb, xs16, pb, pf, gselT, out)
    nc.compile()
    return nc


def _emit(nc, tc, xb, xs16, pb, pf, gselT, out):
    with (
        tc.tile_pool(name="pp", bufs=1) as pp,
        tc.tile_pool(name="ptp", bufs=6) as ptp,
        tc.tile_pool(name="dp", bufs=1, space="DRAM") as dp,
    ):
        # Pin the exp+ln+square table set before any ACT op; every
        # loop/epilogue activation is then satisfied and the only
        # remaining switch (silu) hides under the stats exchange.
        _load_act_set(nc, ACT_SET_MAIN)

        # ---------------- loads (two HWDGE rings in parallel) -----------
        pb_sb = pp.tile([P, NPB], F16)
        nc.scalar.dma_start(out=pb_sb[:, 0:256], in_=pb[:, 0:256])
        xsr = pp.tile([P, NS], F16)
        nc.scalar.dma_start(out=xsr[:, 0:512], in_=xs16[:, 0:512])
        nc.scalar.dma_start(out=pb_sb[:, 256:NPB], in_=pb[:, 256:NPB])
        pf_sb = pp.tile([P, NPF], F32)
        nc.scalar.dma_start(out=pf_sb[:], in_=pf[:])
        gselT_sb = pp.tile([NGROUPS, P], F32)
        nc.scalar.dma_start(out=gselT_sb[:], in_=gselT[:])
        # warm-up collective: aligns core start (prevents a fast core's
        # remote stats write racing a slow core's semaphore clear) and
        # wakes the CC cores in parallel with the prologue DMAs
        warm = pp.tile([1, 2], F32)
        nc.vector.memset(warm[:], 0.0)
        dumc_in = dp.tile([1, 2], F32)
        dumc_out = dp.tile([1, 2], F32)
        nc.sync.dma_start(out=dumc_in[:], in_=warm[:])
        nc.gpsimd.collective_compute(
            "AllReduce", ALU.add,
            replica_groups=[[0, 1, 2, 3, 4, 5, 6, 7]],
            ins=[dumc_in.opt()], outs=[dumc_out.opt()],
        )
        nc.sync.dma_start(out=xsr[:, 512:NS], in_=xs16[:, 512:NS])
        xr = pp.tile([P, N], F16)
        for i in range(8):
            nc.sync.dma_start(out=xr[:, i * 512:(i + 1) * 512],
                              in_=xb[:, i * 512:(i + 1) * 512])
        wq_b = pb_sb[:, 0:128]
        wk_b = pb_sb[:, 128:256]
        wvT_b = pb_sb[:, 256:384]
        woT_b = pb_sb[:, 384:512]
        id16_b = pb_sb[:, 512:640]
        gsel_c = pf_sb[:, 0:NGROUPS]        # scaled by 1/CNT host-side
        gamma_sb = pf_sb[:, NGROUPS:NGROUPS + 1]
        beta_sb = pf_sb[:, NGROUPS + 1:NGROUPS + 2]
        bsel0 = pf_sb[:, NGROUPS + 2:NGROUPS + 3]   # 1.0 iff batch-0 core
        bsel1 = pf_sb[:, NGROUPS + 3:NGROUPS + 4]   # 1.0 iff batch-1 core

        gselT_c = pp.tile([NGROUPS, P], F32)
        nc.vector.tensor_copy(gselT_c[:], gselT_sb[:])
        onesM = pp.tile([P, P], BF16)
        nc.vector.memset(onesM[:], 1.0)
        # Global exp shift: cancels exactly in softmax. Centers the
        # log-rowsum range [21.6, 103.5] inside exp/ln's clean window.
        shift = pp.tile([P, 1], F32)
        nc.vector.memset(shift[:], -62.5)

        stat_sb = pp.tile([P, 2], F32)

        # ------------- projections + attention loop (interleaved) -------
        r_r = pp.tile([P, NS], F16)
        vt_sb = pp.tile([P, NB, P], BF16)
        h_sb = pp.tile([P, NS], F16)
        rsA = pp.tile([P, NS], BF16)
        rsB = pp.tile([P, NS], BF16)
        with tc.tile_pool(name="acc", bufs=1, space="PSUM") as acc:
          with tc.tile_pool(name="stp", bufs=2, space="PSUM") as stp:
            h_ps = acc.tile([P, NS], F32, tag="h")

            # M = Wq^T Wk  -> R = M^T Xs
            at_ps = stp.tile([P, P], F32, tag="st", name="at_ps")
            nc.tensor.matmul(at_ps[:], wq_b, wk_b, start=True, stop=True)
            at_b = pp.tile([P, P], F16)
            nc.vector.tensor_copy(at_b[:], at_ps[:])
            r_ps = stp.tile([P, NS], F32, tag="st", name="r_ps")
            nc.tensor.matmul(r_ps[:, 0:512], at_b[:], xsr[:, 0:512],
                             start=True, stop=True)
            nc.tensor.matmul(r_ps[:, 512:NS], at_b[:], xsr[:, 512:NS],
                             start=True, stop=True)
            nc.vector.tensor_copy(r_r[:, 0:512], r_ps[:, 0:512])
            nc.vector.tensor_copy(r_r[:, 512:NS], r_ps[:, 512:NS])

            def emit_vt(j):
                # V^T chunk j = X_j^T Wv^T directly (one 128-free matmul;
                # the stationary X chunk is the same one the score matmuls
                # use) — replaces the V projection + PE transposes.
                vt_ps = stp.tile([P, P], F32, tag="vt", bufs=2,
                                 name=f"vt_ps{j}")
                nc.tensor.matmul(vt_ps[:], xr[:, j * P:(j + 1) * P], wvT_b,
                                 start=True, stop=True)
                nc.vector.tensor_copy(vt_sb[:, j, :], vt_ps[:])

            def consume(jj, ptj):
                first = jj == 0
                last = jj == NB - 1
                nc.tensor.matmul(h_ps[:, 0:512], vt_sb[:, jj, :], ptj[:, 0:512],
                                 start=first, stop=last)
                nc.tensor.matmul(h_ps[:, 512:NS], vt_sb[:, jj, :], ptj[:, 512:NS],
                                 start=first, stop=last)

            def rs_add(jj, ptj):
                # dual bf16 accumulators: 2x DVE mode, halved error depth
                dst = rsA if jj % 2 == 0 else rsB
                if jj < 2:
                    nc.vector.tensor_copy(dst[:], ptj[:])
                else:
                    nc.vector.tensor_add(dst[:], dst[:], ptj[:])

            # scores start immediately (need only xr chunk 0 + R); each
            # iteration also emits its V^T chunk; PV matmuls lag two
            # iterations, the DVE row-sum adds lag one.
            pts = []
            for j in range(NB):
                st_ps = stp.tile([P, NS], F32, tag="st", name=f"st_ps{j}")
                lhs = xr[:, j * P:(j + 1) * P]
                pt = ptp.tile([P, NS], BF16, tag="pt", name=f"pt{j}")
                if j < 2:
                    # halves, exp emitted immediately after its half-matmul:
                    # the cross-engine counting semaphore then excludes the
                    # second half, so the first exps start on R's first half
                    nc.tensor.matmul(st_ps[:, 0:512], lhs, r_r[:, 0:512],
                                     start=True, stop=True)
                    nc.scalar.activation(pt[:, 0:512], st_ps[:, 0:512],
                                         AF.Exp, bias=shift[:])
                    nc.tensor.matmul(st_ps[:, 512:NS], lhs, r_r[:, 512:NS],
                                     start=True, stop=True)
                    nc.scalar.activation(pt[:, 512:NS], st_ps[:, 512:NS],
                                         AF.Exp, bias=shift[:])
                else:
                    nc.tensor.matmul(st_ps[:, 0:512], lhs, r_r[:, 0:512],
                                     start=True, stop=True)
                    nc.tensor.matmul(st_ps[:, 512:NS], lhs, r_r[:, 512:NS],
                                     start=True, stop=True)
                    nc.scalar.activation(pt[:], st_ps[:], AF.Exp,
                                         bias=shift[:])
                pts.append(pt)
                # V^T chunk after the score matmuls: it shares their
                # stationary X chunk but must not delay the exp feed; its
                # consumer is two iterations away.
                emit_vt(j)
                if j >= 2:
                    consume(j - 2, pts[j - 2])
                if j >= 1:
                    rs_add(j - 1, pts[j - 1])
            rs_add(NB - 1, pts[NB - 1])

            # broadcast-fold both accumulators with an all-ones stationary,
            # summing them in PSUM: rb[p, n] = rowsum[n] on every partition.
            # Emitted BEFORE the final PV consumes: the ln/exp reciprocal
            # chain it feeds is longer than the consumes' slack (the h
            # multiply waits on the reciprocal, not the other way around).
            rb_ps = stp.tile([P, NS], F32, tag="st", name="rb_ps")
            nc.tensor.matmul(rb_ps[:, 0:512], onesM[:], rsA[:, 0:512],
                             start=True, stop=False)
            nc.tensor.matmul(rb_ps[:, 0:512], onesM[:], rsB[:, 0:512],
                             start=False, stop=True)
            nc.tensor.matmul(rb_ps[:, 512:NS], onesM[:], rsA[:, 512:NS],
                             start=True, stop=False)
            nc.tensor.matmul(rb_ps[:, 512:NS], onesM[:], rsB[:, 512:NS],
                             start=False, stop=True)
            for jj in (NB - 2, NB - 1):
                consume(jj, pts[jj])

            # 1/rowsum = exp(-ln(rowsum)): both in the pinned table set;
            # covers the whole fp32 range unlike the ACT reciprocal.
            lnr = pp.tile([P, NS], F32)
            nc.scalar.activation(lnr[:], rb_ps[:], AF.Ln)
            rbinv = pp.tile([P, NS], F32)
            nc.scalar.activation(rbinv[:], lnr[:], AF.Exp, scale=-1.0)

          # ------- output projection + residual + GroupNorm + SiLU -------
          # stp closed (banks freed); h_ps still live for the h-multiplies.
          with tc.tile_pool(name="ep", bufs=1, space="PSUM") as ep:
            # y = Wo h + x accumulated entirely in PSUM: the residual opens
            # each half's accumulation (xsr-only dependency, runs during the
            # reciprocal window); per-half interleave keeps every wait tight
            # (wo0 after hmul0 only; stats right behind their half).
            a_ps = ep.tile([P, NS], F32, tag="a")
            nc.tensor.matmul(a_ps[:, 0:512], id16_b, xsr[:, 0:512],
                             start=True, stop=False)
            nc.tensor.matmul(a_ps[:, 512:NS], id16_b, xsr[:, 512:NS],
                             start=True, stop=False)
            nc.vector.tensor_mul(h_sb[:, 0:512], h_ps[:, 0:512],
                                 rbinv[:, 0:512])
            nc.tensor.matmul(a_ps[:, 0:512], woT_b, h_sb[:, 0:512],
                             start=False, stop=True)
            nc.vector.tensor_mul(h_sb[:, 512:NS], h_ps[:, 512:NS],
                                 rbinv[:, 512:NS])
            nc.tensor.matmul(a_ps[:, 512:NS], woT_b, h_sb[:, 512:NS],
                             start=False, stop=True)

            # per-channel partial stats over the local 1024 columns, read
            # straight from PSUM; halves so DVE reduce and ACT square overlap
            hsum = pp.tile([P, 2], F32)
            nc.vector.reduce_sum(hsum[:, 0:1], a_ps[:, 0:512], axis=AX.X)
            sq_sb = pp.tile([P, NS], F32)
            nc.scalar.activation(sq_sb[:, 0:512], a_ps[:, 0:512], AF.Square,
                                 accum_out=hsum[:, 1:2])
            hsum2 = pp.tile([P, 2], F32)
            nc.vector.reduce_sum(hsum2[:, 0:1], a_ps[:, 512:NS], axis=AX.X)
            nc.scalar.activation(sq_sb[:, 512:NS], a_ps[:, 512:NS], AF.Square,
                                 accum_out=hsum2[:, 1:2])
            nc.vector.tensor_add(stat_sb[:], hsum[:], hsum2[:])

            # ONE 8-rank AllReduce on a batch-masked [128, 4] payload:
            # cols 0-1 carry this core's stats if it is a batch-0 core,
            # cols 2-3 if batch-1. Two concurrent 4-rank group collectives
            # serialize on the CC machinery (~15us extra for the second
            # group); a single 8-rank op avoids that. Each core selects its
            # batch's half post-reduce. Silu table set preloads in flight.
            ms_sb = pp.tile([P, 4], F32)
            nc.vector.tensor_scalar(out=ms_sb[:, 0:2], in0=stat_sb[:],
                                    scalar1=bsel0, scalar2=None,
                                    op0=ALU.mult)
            nc.vector.tensor_scalar(out=ms_sb[:, 2:4], in0=stat_sb[:],
                                    scalar1=bsel1, scalar2=None,
                                    op0=ALU.mult)
            d_st1 = dp.tile([P, 4], F32)
            d_st2 = dp.tile([P, 4], F32)
            nc.sync.dma_start(out=d_st1[:], in_=ms_sb[:])
            nc.gpsimd.collective_compute(
                "AllReduce", ALU.add,
                replica_groups=[[0, 1, 2, 3, 4, 5, 6, 7]],
                ins=[d_st1.opt()], outs=[d_st2.opt()],
            )
            ast_sb = pp.tile([P, 4], F32)
            nc.sync.dma_start(out=ast_sb[:], in_=d_st2[:])

            # fold channels -> groups FIRST (linear, so fold-then-select is
            # exact); gsel carries 1/CNT so this yields [mean, E[y^2]] per
            # group for both batches, then the batch half is selected on
            # tiny [8, 2] tiles (bsel columns are constant per partition)
            gs_ps = ep.tile([NGROUPS, 4], F32, tag="gs")
            nc.tensor.matmul(gs_ps[:], gsel_c, ast_sb[:], start=True, stop=True)
            s0 = pp.tile([NGROUPS, 2], F32)
            nc.vector.tensor_scalar(out=s0[:], in0=gs_ps[:, 0:2],
                                    scalar1=bsel0[0:NGROUPS, :], scalar2=None,
                                    op0=ALU.mult)
            mg = pp.tile([NGROUPS, 2], F32)
            nc.vector.tensor_scalar(out=mg[:], in0=gs_ps[:, 2:4],
                                    scalar1=bsel1[0:NGROUPS, :], scalar2=s0[:, 0:1],
                                    op0=ALU.mult, op1=ALU.bypass)
            nc.vector.tensor_add(mg[:], mg[:], s0[:])
            msq = pp.tile([NGROUPS, 1], F32)
            nc.vector.tensor_mul(msq[:], mg[:, 0:1], mg[:, 0:1])
            var8 = pp.tile([NGROUPS, 1], F32)
            nc.vector.tensor_sub(var8[:], mg[:, 1:2], msq[:])
            # rstd = 1/sqrt(var + eps) = exp(-0.5 ln(var + eps)): the
            # pinned set 6 (exp+ln) is still resident after the AR, so this
            # is two tiny ACT ops writing straight into gval; the silu's
            # table switch (the only one) then hides under the z chain.
            ve8 = pp.tile([NGROUPS, 1], F32)
            nc.vector.tensor_scalar_add(ve8[:], in0=var8[:], scalar1=EPS)
            lnv = pp.tile([NGROUPS, 1], F32)
            nc.scalar.activation(lnv[:], ve8[:], AF.Ln)
            gval = pp.tile([NGROUPS, 2], F32)
            nc.vector.tensor_copy(gval[:, 0:1], mg[:, 0:1])
            nc.scalar.activation(gval[:, 1:2], lnv[:], AF.Exp, scale=-0.5)

            # broadcast group stats back to channels: [128, 2] = G @ gval
            pc_ps = ep.tile([P, 2], F32, tag="pc")
            nc.tensor.matmul(pc_ps[:], gselT_c[:], gval[:], start=True, stop=True)
            pc_sb = pp.tile([P, 2], F32)
            nc.vector.tensor_copy(pc_sb[:], pc_ps[:])

            # fuse (y - mean)*rstd*gamma + beta into one pass:
            # A = rstd*gamma, B = beta - mean*A, z = y*A + B
            A_sb = pp.tile([P, 1], F32)
            nc.vector.tensor_mul(A_sb[:], pc_sb[:, 1:2], gamma_sb)
            t_sb = pp.tile([P, 1], F32)
            nc.vector.tensor_mul(t_sb[:], pc_sb[:, 0:1], A_sb[:])
            B_sb = pp.tile([P, 1], F32)
            nc.vector.tensor_sub(B_sb[:], beta_sb, t_sb[:])
            # affine + silu + store fully by halves: z's second half runs
            # on the DVE while ACT processes the first silu, and each
            # half's store DMA overlaps the next activation
            z_sb = pp.tile([P, NS], F32)
            o_sb = pp.tile([P, NS], F32)
            nc.vector.tensor_scalar(out=z_sb[:, 0:512], in0=a_ps[:, 0:512],
                                    scalar1=A_sb[:], scalar2=B_sb[:],
                                    op0=ALU.mult, op1=ALU.add)
            nc.scalar.activation(o_sb[:, 0:512], z_sb[:, 0:512], AF.Silu)
            nc.vector.tensor_scalar(out=z_sb[:, 512:NS], in0=a_ps[:, 512:NS],
                                    scalar1=A_sb[:], scalar2=B_sb[:],
                                    op0=ALU.mult, op1=ALU.add)
            nc.sync.dma_start(out=out[:, 0:512], in_=o_sb[:, 0:512])
            nc.scalar.activation(o_sb[:, 512:NS], z_sb[:, 512:NS], AF.Silu)
            nc.sync.dma_start(out=out[:, 512:NS], in_=o_sb[:, 512:NS])


_NC_CACHE = None


def _get_nc():
    global _NC_CACHE
    if _NC_CACHE is None:
        _NC_CACHE = _build_nc()
    return _NC_CACHE


def make_in_maps(x, Wq, Wk, Wv, Wo, gamma, beta):
    x = np.asarray(x, dtype=np.float32)
    B, C = x.shape[0], x.shape[1]
    xf = np.ascontiguousarray(x.reshape(B, C, -1))
    xf16 = xf.astype(np.float16)
    Wq = np.asarray(Wq, dtype=np.float32)
    Wk = np.asarray(Wk, dtype=np.float32)
    WvT = np.asarray(Wv, dtype=np.float32).T
    WoT = np.asarray(Wo, dtype=np.float32).T
    g = np.asarray(gamma, dtype=np.float32).reshape(P, 1)
    b = np.asarray(beta, dtype=np.float32).reshape(P, 1)
    gs = np.zeros((P, NGROUPS), dtype=np.float32)
    gs[np.arange(P), np.arange(P) // (P // NGROUPS)] = 1.0
    gsT = np.ascontiguousarray(gs.T)
    ident = np.eye(P, dtype=np.float32)
    pbm = np.ascontiguousarray(
        np.concatenate([Wq, Wk, WvT, WoT, ident], axis=1)).astype(np.float16)
    assert pbm.shape == (P, NPB)
    in_maps = []
    for core in range(NCORES):
        bi, s = core // 4, core % 4
        bsel = np.zeros((P, 2), dtype=np.float32)
        bsel[:, bi] = 1.0
        pfm = np.ascontiguousarray(
            np.concatenate([gs * (1.0 / CNT), g, b, bsel],
                           axis=1)).astype(np.float32)
        assert pfm.shape == (P, NPF)
        in_maps.append({
            "xb": xf16[bi],
            "xs16": np.ascontiguousarray(xf16[bi][:, s * NS:(s + 1) * NS]),
            "pb": pbm, "pf": pfm, "gselT": gsT,
        })
    return in_maps


def assemble(results, spatial=(16, 16, 16)):
    y = np.empty((2, P, N), dtype=np.float32)
    for core in range(NCORES):
        bi, s = core // 4, core % 4
        y[bi][:, s * NS:(s + 1) * NS] = results[core]["out"]
    return y.reshape(2, P, *spatial)


def kernel(x, Wq, Wk, Wv, Wo, gamma, beta):
    nc = _get_nc()
    in_maps = make_in_maps(x, Wq, Wk, Wv, Wo, gamma, beta)
    res = run_bass_kernel_spmd(nc, in_maps, list(range(NCORES)))
    return assemble(res.results, spatial=tuple(np.asarray(x).shape[2:]))



# revision 10
# speedup vs baseline: 1.6005x; 1.6005x over previous
"""Trainium2 Bass kernel for nn_Attention_73718818669284.

Reference computation (per batch b of 2, C=128 channels, N=4096 spatial):
    q = Wq x, k = Wk x, v = Wv x           (1x1 conv == channel matmul)
    w = softmax(q^T k, axis=-1)            ([N, N] attention)
    h = Wo (v w^T)
    y = x + h
    out = SiLU(GroupNorm8(y) * gamma + beta)

Sharding: 8 cores = 2 batches x 4 column-slices of N (1024 each). The
host rotates each core's xb by its slice offset so every core's slice
sits at columns 0:1024 (the m-chunk order of the attention loop is a
rotation, which softmax/PV sums are invariant to); this makes the
program uniform and drops a separate slice load.

GroupNorm statistics are combined with ONE 8-rank AllReduce on a
batch-masked, group-folded [8, 4] payload (each core contributes its
group stats in its batch's column pair and selects its half
post-reduce). The fold to groups happens BEFORE the exchange (linear,
so exact) which shrinks the payload and removes the post-AR fold
matmul from the critical path. There is deliberately NO warm-up
collective: the CC stream serializes its ops, so a warm-up AR executes
immediately before the stats AR and adds its full ~8us to the critical
path (measured); core start alignment is already provided by the
runtime's NEFF-entry CC barrier that comes with has_collectives.
(A remote-DMA peer-exchange without the CC machinery was tried and
hard-crashes this environment's runtime — NRT_EXEC_UNIT_UNRECOVERABLE
on any SWDGE remote descriptor — so the collective stays.)

Per-core algorithm (transposed-score layout -> no PE transposes of P):
    M   = Wq^T Wk                     (one 128x128 matmul, fp16)
    R   = M^T X_s                     ([128, 1024] fp16, folds q-projection)
    S^T chunk j = X[:,128j:]^T R      ([128m, 1024n]; scores, fp16 inputs)
    P^T = exp(S^T + shift)            (bf16; shift cancels in softmax)
    rowsum = sum_m P^T[m, n]          (DVE bf16 dual accumulators)
    h_un = V P = sum_j VT_j^T PT_j    (VT_j = X_j^T Wv^T directly, bf16)
    h = h_un * (1/rowsum)             (1/r = exp(-ln r) on ACT, set 6)
    y = Wo h + x_s ; stats exchange; GroupNorm; SiLU.

Matmul dtypes: the score path (X, M, R, Wo/h) runs in fp16 (10-bit
mantissa keeps score errors ~1e-3; bf16 scores measure 3e-2 rel err),
the P-side (exp output, V, rowsums) in bf16 (fp16 would overflow at
e^41). Both stream 1 cycle/row with fast weight load; the fp32 moving
path measures ~2x slower. The ACT table set 6 (exp+ln+square) is pinned
at kernel entry; the only switch (silu) hides under the stats exchange.
GroupNorm rstd is exp(-0.5 ln(var+eps)) on ACT (set 6 resident).
"""

import numpy as np

import concourse.bass as bass
import concourse.tile as tile
from concourse import bacc, mybir
from concourse.bass_utils import run_bass_kernel_spmd

F32 = mybir.dt.float32
F16 = mybir.dt.float16
BF16 = mybir.dt.bfloat16
NPBF16 = mybir.dt.np(mybir.dt.bfloat16)
AF = mybir.ActivationFunctionType
ALU = mybir.AluOpType
AX = mybir.AxisListType

P = 128          # channels / partitions
N = 4096         # spatial size (16*16*16)
NS = 1024        # per-core slice of N
NB = N // P      # 32 m-chunks
NCORES = 8
NGROUPS = 8
EPS = 1e-5
CNT = (P // NGROUPS) * N   # elements per group per batch = 16 * 4096
NPB = 5 * P                # fp16 params width (wq|wk|wvT|woT|ident)
NPF = NGROUPS + 4          # fp32 params width (gsel/CNT | gamma | beta | bsel0 | bsel1)
ACT_SET_MAIN = 6           # natural_log_exp_and_others: exp + ln + square


def _load_act_set(nc, set_id):
    return nc.scalar.add_instruction(
        mybir.InstLoadActFuncSet(
            name=nc.get_next_instruction_name(),
            ins=[], outs=[],
            act_func_set_id=set_id,
        )
    )


def _build_nc():
    nc = bacc.Bacc("TRN2", target_bir_lowering=False, debug=False,
                   num_devices=NCORES)

    xb = nc.declare_dram_parameter("xb", [P, N], F16, isOutput=False)
    pb = nc.declare_dram_parameter("pb", [P, NPB], F16, isOutput=False)
    pf = nc.declare_dram_parameter("pf", [P, NPF], F32, isOutput=False)
    gselT = nc.declare_dram_parameter("gselT", [NGROUPS, P], BF16,
                                      isOutput=False)
    out = nc.declare_dram_parameter("out", [P, NS], F32, isOutput=True)

    with tile.TileContext(nc) as tc:
        _emit(nc, tc, xb, pb, pf, gselT, out)
    nc.compile()
    return nc


def _emit(nc, tc, xb, pb, pf, gselT, out):
    with (
        tc.tile_pool(name="pp", bufs=1) as pp,
        tc.tile_pool(name="ptp", bufs=6) as ptp,
        tc.tile_pool(name="dp", bufs=1, space="DRAM") as dp,
    ):
        # Pin the exp+ln+square table set before any ACT op; every
        # loop/epilogue activation is then satisfied and the only
        # remaining switch (silu) hides under the stats exchange.
        _load_act_set(nc, ACT_SET_MAIN)

        # ---------------- loads (two HWDGE rings in parallel) -----------
        pb_sb = pp.tile([P, NPB], F16)
        nc.scalar.dma_start(out=pb_sb[:, 0:256], in_=pb[:, 0:256])
        xr = pp.tile([P, N], F16)
        nc.sync.dma_start(out=xr[:, 0:512], in_=xb[:, 0:512])
        nc.scalar.dma_start(out=pb_sb[:, 256:NPB], in_=pb[:, 256:NPB])
        nc.sync.dma_start(out=xr[:, 512:1024], in_=xb[:, 512:1024])
        pf_sb = pp.tile([P, NPF], F32)
        nc.scalar.dma_start(out=pf_sb[:], in_=pf[:])
        gselT_sb = pp.tile([NGROUPS, P], BF16)
        nc.scalar.dma_start(out=gselT_sb[:], in_=gselT[:])
        for i in range(2, 8):
            nc.sync.dma_start(out=xr[:, i * 512:(i + 1) * 512],
                              in_=xb[:, i * 512:(i + 1) * 512])
        xsr = xr[:, 0:NS]   # this core's slice (host-rotated to the front)
        wq_b = pb_sb[:, 0:128]
        wk_b = pb_sb[:, 128:256]
        wvT_b = pb_sb[:, 256:384]
        woT_b = pb_sb[:, 384:512]
        id16_b = pb_sb[:, 512:640]
        gsel_c = pf_sb[:, 0:NGROUPS]        # scaled by 1/CNT host-side
        gamma_sb = pf_sb[:, NGROUPS:NGROUPS + 1]
        beta_sb = pf_sb[:, NGROUPS + 1:NGROUPS + 2]
        bsel0 = pf_sb[:, NGROUPS + 2:NGROUPS + 3]   # 1.0 iff batch-0 core
        bsel1 = pf_sb[:, NGROUPS + 3:NGROUPS + 4]   # 1.0 iff batch-1 core

        onesM = pp.tile([P, P], BF16)
        nc.vector.memset(onesM[:], 1.0)
        # Global exp shift: cancels exactly in softmax. Centers the
        # log-rowsum range [21.6, 103.5] inside exp/ln's clean window.
        shift = pp.tile([P, 1], F32)
        nc.vector.memset(shift[:], -62.5)

        stat_sb = pp.tile([P, 2], F32)

        # ------------- projections + attention loop (interleaved) -------
        r_r = pp.tile([P, NS], F16)
        vt_sb = pp.tile([P, NB, P], BF16)
        h_sb = pp.tile([P, NS], F16)
        rsA = pp.tile([P, NS], BF16)
        rsB = pp.tile([P, NS], BF16)
        with tc.tile_pool(name="acc", bufs=1, space="PSUM") as acc:
          with tc.tile_pool(name="stp", bufs=2, space="PSUM") as stp:
            h_ps = acc.tile([P, NS], F32, tag="h")

            # M = Wq^T Wk  -> R = M^T Xs
            at_ps = stp.tile([P, P], F32, tag="st", name="at_ps")
            nc.tensor.matmul(at_ps[:], wq_b, wk_b, start=True, stop=True)
            at_b = pp.tile([P, P], F16)
            nc.vector.tensor_copy(at_b[:], at_ps[:])
            r_ps = stp.tile([P, NS], F32, tag="st", name="r_ps")
            nc.tensor.matmul(r_ps[:, 0:512], at_b[:], xsr[:, 0:512],
                             start=True, stop=True)
            nc.tensor.matmul(r_ps[:, 512:NS], at_b[:], xsr[:, 512:NS],
                             start=True, stop=True)
            nc.vector.tensor_copy(r_r[:, 0:512], r_ps[:, 0:512])
            nc.vector.tensor_copy(r_r[:, 512:NS], r_ps[:, 512:NS])

            def emit_vt(j):
                # V^T chunk j = X_j^T Wv^T directly (one 128-free matmul;
                # the stationary X chunk is the same one the score matmuls
                # use) — replaces the V projection + PE transposes.
                vt_ps = stp.tile([P, P], F32, tag="vt", bufs=2,
                                 name=f"vt_ps{j}")
                nc.tensor.matmul(vt_ps[:], xr[:, j * P:(j + 1) * P], wvT_b,
                                 start=True, stop=True)
                nc.vector.tensor_copy(vt_sb[:, j, :], vt_ps[:])

            def consume(jj, ptj):
                first = jj == 0
                last = jj == NB - 1
                nc.tensor.matmul(h_ps[:, 0:512], vt_sb[:, jj, :], ptj[:, 0:512],
                                 start=first, stop=last)
                nc.tensor.matmul(h_ps[:, 512:NS], vt_sb[:, jj, :], ptj[:, 512:NS],
                                 start=first, stop=last)

            def rs_add(jj, ptj):
                # dual bf16 accumulators: 2x DVE mode, halved error depth
                dst = rsA if jj % 2 == 0 else rsB
                if jj < 2:
                    nc.vector.tensor_copy(dst[:], ptj[:])
                else:
                    nc.vector.tensor_add(dst[:], dst[:], ptj[:])

            # scores start immediately (need only xr chunk 0 + R); each
            # iteration also emits its V^T chunk; PV matmuls lag two
            # iterations, the DVE row-sum adds lag one.
            pts = []
            for j in range(NB):
                st_ps = stp.tile([P, NS], F32, tag="st", name=f"st_ps{j}")
                lhs = xr[:, j * P:(j + 1) * P]
                pt = ptp.tile([P, NS], BF16, tag="pt", name=f"pt{j}")
                if j < 2:
                    # halves, exp emitted immediately after its half-matmul:
                    # the cross-engine counting semaphore then excludes the
                    # second half, so the first exps start on R's first half
                    nc.tensor.matmul(st_ps[:, 0:512], lhs, r_r[:, 0:512],
                                     start=True, stop=True)
                    nc.scalar.activation(pt[:, 0:512], st_ps[:, 0:512],
                                         AF.Exp, bias=shift[:])
                    nc.tensor.matmul(st_ps[:, 512:NS], lhs, r_r[:, 512:NS],
                                     start=True, stop=True)
                    nc.scalar.activation(pt[:, 512:NS], st_ps[:, 512:NS],
                                         AF.Exp, bias=shift[:])
                else:
                    nc.tensor.matmul(st_ps[:, 0:512], lhs, r_r[:, 0:512],
                                     start=True, stop=True)
                    nc.tensor.matmul(st_ps[:, 512:NS], lhs, r_r[:, 512:NS],
                                     start=True, stop=True)
                    nc.scalar.activation(pt[:], st_ps[:], AF.Exp,
                                         bias=shift[:])
                pts.append(pt)
                # V^T chunk after the score matmuls: it shares their
                # stationary X chunk but must not delay the exp feed; its
                # consumer is two iterations away.
                emit_vt(j)
                if j >= 2:
                    consume(j - 2, pts[j - 2])
                if j >= 1:
                    rs_add(j - 1, pts[j - 1])
            rs_add(NB - 1, pts[NB - 1])

            # broadcast-fold both accumulators with an all-ones stationary,
            # summing them in PSUM: rb[p, n] = rowsum[n] on every partition.
            # Emitted BEFORE the final PV consumes: the ln/exp reciprocal
            # chain it feeds is longer than the consumes' slack (the h
            # multiply waits on the reciprocal, not the other way around).
            rb_ps = stp.tile([P, NS], F32, tag="st", name="rb_ps")
            nc.tensor.matmul(rb_ps[:, 0:512], onesM[:], rsA[:, 0:512],
                             start=True, stop=False)
            nc.tensor.matmul(rb_ps[:, 0:512], onesM[:], rsB[:, 0:512],
                             start=False, stop=True)
            nc.tensor.matmul(rb_ps[:, 512:NS], onesM[:], rsA[:, 512:NS],
                             start=True, stop=False)
            nc.tensor.matmul(rb_ps[:, 512:NS], onesM[:], rsB[:, 512:NS],
                             start=False, stop=True)
            for jj in (NB - 2, NB - 1):
                consume(jj, pts[jj])

            # 1/rowsum = exp(-ln(rowsum)): both in the pinned table set;
            # covers the whole fp32 range unlike the ACT reciprocal.
            lnr = pp.tile([P, NS], F32)
            nc.scalar.activation(lnr[:], rb_ps[:], AF.Ln)
            rbinv = pp.tile([P, NS], F32)
            nc.scalar.activation(rbinv[:], lnr[:], AF.Exp, scale=-1.0)

          # ------- output projection + residual + GroupNorm + SiLU -------
          # stp closed (banks freed); h_ps still live for the h-multiplies.
          with tc.tile_pool(name="ep", bufs=1, space="PSUM") as ep:
            # y = Wo h + x accumulated entirely in PSUM: the residual opens
            # each half's accumulation (xsr-only dependency, runs during the
            # reciprocal window); per-half interleave keeps every wait tight
            # (wo0 after hmul0 only; stats right behind their half).
            a_ps = ep.tile([P, NS], F32, tag="a")
            nc.tensor.matmul(a_ps[:, 0:512], id16_b, xsr[:, 0:512],
                             start=True, stop=False)
            nc.tensor.matmul(a_ps[:, 512:NS], id16_b, xsr[:, 512:NS],
                             start=True, stop=False)
            nc.vector.tensor_mul(h_sb[:, 0:512], h_ps[:, 0:512],
                                 rbinv[:, 0:512])
            nc.tensor.matmul(a_ps[:, 0:512], woT_b, h_sb[:, 0:512],
                             start=False, stop=True)
            nc.vector.tensor_mul(h_sb[:, 512:NS], h_ps[:, 512:NS],
                                 rbinv[:, 512:NS])
            nc.tensor.matmul(a_ps[:, 512:NS], woT_b, h_sb[:, 512:NS],
                             start=False, stop=True)

            # per-channel partial stats over the local 1024 columns, read
            # straight from PSUM. Emission order puts the two DVE reduces
            # back-to-back so they run concurrently with the ACT squares.
            hsum = pp.tile([P, 2], F32)
            hsum2 = pp.tile([P, 2], F32)
            sq_sb = pp.tile([P, NS], F32)
            nc.vector.reduce_sum(hsum[:, 0:1], a_ps[:, 0:512], axis=AX.X)
            nc.scalar.activation(sq_sb[:, 0:512], a_ps[:, 0:512], AF.Square,
                                 accum_out=hsum[:, 1:2])
            nc.vector.reduce_sum(hsum2[:, 0:1], a_ps[:, 512:NS], axis=AX.X)
            nc.scalar.activation(sq_sb[:, 512:NS], a_ps[:, 512:NS], AF.Square,
                                 accum_out=hsum2[:, 1:2])
            nc.vector.tensor_add(stat_sb[:], hsum[:], hsum2[:])

            # fold channels -> groups BEFORE the exchange (linear, so
            # fold-then-reduce is exact); gsel carries 1/CNT so rows 0:8
            # hold [mean, E[y^2]] partials. Mask into this core's batch
            # columns; the AllReduce payload is a tiny [8, 4].
            gf_ps = ep.tile([NGROUPS, 2], F32, tag="gs")
            nc.tensor.matmul(gf_ps[:], gsel_c, stat_sb[:], start=True,
                             stop=True)
            ms_sb = pp.tile([NGROUPS, 4], F32)
            nc.vector.tensor_scalar(out=ms_sb[:, 0:2], in0=gf_ps[:],
                                    scalar1=bsel0[0:NGROUPS, :], scalar2=None,
                                    op0=ALU.mult)
            nc.vector.tensor_scalar(out=ms_sb[:, 2:4], in0=gf_ps[:],
                                    scalar1=bsel1[0:NGROUPS, :], scalar2=None,
                                    op0=ALU.mult)
            d_st1 = dp.tile([NGROUPS, 4], F32)
            d_st2 = dp.tile([NGROUPS, 4], F32)
            nc.sync.dma_start(out=d_st1[:], in_=ms_sb[:])
            nc.gpsimd.collective_compute(
                "AllReduce", ALU.add,
                replica_groups=[[0, 1, 2, 3, 4, 5, 6, 7]],
                ins=[d_st1.opt()], outs=[d_st2.opt()],
            )
            ast_sb = pp.tile([NGROUPS, 4], F32)
            nc.sync.dma_start(out=ast_sb[:], in_=d_st2[:])

            # select this batch's column pair on tiny [8, 2] tiles (bsel
            # columns are constant per partition). Silu table set preloads
            # on ACT while the exchange is in flight.
            s0 = pp.tile([NGROUPS, 2], F32)
            nc.vector.tensor_scalar(out=s0[:], in0=ast_sb[:, 0:2],
                                    scalar1=bsel0[0:NGROUPS, :], scalar2=None,
                                    op0=ALU.mult)
            mg = pp.tile([NGROUPS, 2], F32)
            nc.vector.tensor_scalar(out=mg[:], in0=ast_sb[:, 2:4],
                                    scalar1=bsel1[0:NGROUPS, :], scalar2=s0[:, 0:1],
                                    op0=ALU.mult, op1=ALU.bypass)
            nc.vector.tensor_add(mg[:], mg[:], s0[:])
            msq = pp.tile([NGROUPS, 1], F32)
            nc.vector.tensor_mul(msq[:], mg[:, 0:1], mg[:, 0:1])
            var8 = pp.tile([NGROUPS, 1], F32)
            nc.vector.tensor_sub(var8[:], mg[:, 1:2], msq[:])
            # rstd = 1/sqrt(var + eps) = exp(-0.5 ln(var + eps)): the
            # pinned set 6 (exp+ln) is still resident after the AR, so this
            # is two tiny ACT ops writing straight into gval; the silu's
            # table switch (the only one) then hides under the z chain.
            ve8 = pp.tile([NGROUPS, 1], F32)
            nc.vector.tensor_scalar_add(ve8[:], in0=var8[:], scalar1=EPS)
            lnv = pp.tile([NGROUPS, 1], F32)
            nc.scalar.activation(lnv[:], ve8[:], AF.Ln)
            gval = pp.tile([NGROUPS, 2], BF16)
            nc.vector.tensor_copy(gval[:, 0:1], mg[:, 0:1])
            nc.scalar.activation(gval[:, 1:2], lnv[:], AF.Exp, scale=-0.5)

            # broadcast group stats back to channels: [128, 2] = G @ gval
            # (bf16 stationary+moving: single fast weight load; gsel values
            # are 0/1 and mean/rstd tolerate bf16's 0.4% step here)
            pc_ps = ep.tile([P, 2], F32, tag="pc")
            nc.tensor.matmul(pc_ps[:], gselT_sb[:], gval[:], start=True,
                             stop=True)
            pc_sb = pp.tile([P, 2], F32)
            nc.vector.tensor_copy(pc_sb[:], pc_ps[:])

            # fuse (y - mean)*rstd*gamma + beta into one pass:
            # A = rstd*gamma, B = beta - mean*A, z = y*A + B
            A_sb = pp.tile([P, 1], F32)
            nc.vector.tensor_mul(A_sb[:], pc_sb[:, 1:2], gamma_sb)
            t_sb = pp.tile([P, 1], F32)
            nc.vector.tensor_mul(t_sb[:], pc_sb[:, 0:1], A_sb[:])
            B_sb = pp.tile([P, 1], F32)
            nc.vector.tensor_sub(B_sb[:], beta_sb, t_sb[:])
            # affine + silu + store fully by halves: z's second half runs
            # on the DVE while ACT processes the first silu, and each
            # half's store DMA overlaps the next activation
            z_sb = pp.tile([P, NS], F32)
            o_sb = pp.tile([P, NS], F32)
            nc.vector.tensor_scalar(out=z_sb[:, 0:512], in0=a_ps[:, 0:512],
                                    scalar1=A_sb[:], scalar2=B_sb[:],
                                    op0=ALU.mult, op1=ALU.add)
            nc.scalar.activation(o_sb[:, 0:512], z_sb[:, 0:512], AF.Silu)
            nc.vector.tensor_scalar(out=z_sb[:, 512:NS], in0=a_ps[:, 512:NS],
                                    scalar1=A_sb[:], scalar2=B_sb[:],
                                    op0=ALU.mult, op1=ALU.add)
            nc.sync.dma_start(out=out[:, 0:512], in_=o_sb[:, 0:512])
            nc.scalar.activation(o_sb[:, 512:NS], z_sb[:, 512:NS], AF.Silu)
            nc.sync.dma_start(out=out[:, 512:NS], in_=o_sb[:, 512:NS])


_NC_CACHE = None


def _get_nc():
    global _NC_CACHE
    if _NC_CACHE is None:
        _NC_CACHE = _build_nc()
    return _NC_CACHE


def make_in_maps(x, Wq, Wk, Wv, Wo, gamma, beta):
    x = np.asarray(x, dtype=np.float32)
    B, C = x.shape[0], x.shape[1]
    xf = np.ascontiguousarray(x.reshape(B, C, -1))
    xf16 = xf.astype(np.float16)
    Wq = np.asarray(Wq, dtype=np.float32)
    Wk = np.asarray(Wk, dtype=np.float32)
    WvT = np.asarray(Wv, dtype=np.float32).T
    WoT = np.asarray(Wo, dtype=np.float32).T
    g = np.asarray(gamma, dtype=np.float32).reshape(P, 1)
    b = np.asarray(beta, dtype=np.float32).reshape(P, 1)
    gs = np.zeros((P, NGROUPS), dtype=np.float32)
    gs[np.arange(P), np.arange(P) // (P // NGROUPS)] = 1.0
    gsT = np.ascontiguousarray(gs.T).astype(NPBF16)
    ident = np.eye(P, dtype=np.float32)
    pbm = np.ascontiguousarray(
        np.concatenate([Wq, Wk, WvT, WoT, ident], axis=1)).astype(np.float16)
    assert pbm.shape == (P, NPB)
    in_maps = []
    for core in range(NCORES):
        bi, s = core // 4, core % 4
        bsel = np.zeros((P, 2), dtype=np.float32)
        bsel[:, bi] = 1.0
        pfm = np.ascontiguousarray(
            np.concatenate([gs * (1.0 / CNT), g, b, bsel],
                           axis=1)).astype(np.float32)
        assert pfm.shape == (P, NPF)
        # rotate this core's slice to the front: column order of the
        # attention m-loop is irrelevant (softmax/PV sum over all m)
        xrot = np.roll(xf16[bi], -s * NS, axis=1)
        in_maps.append({
            "xb": np.ascontiguousarray(xrot),
            "pb": pbm, "pf": pfm, "gselT": gsT,
        })
    return in_maps


def assemble(results, spatial=(16, 16, 16)):
    y = np.empty((2, P, N), dtype=np.float32)
    for core in range(NCORES):
        bi, s = core // 4, core % 4
        y[bi][:, s * NS:(s + 1) * NS] = results[core]["out"]
    return y.reshape(2, P, *spatial)


def kernel(x, Wq, Wk, Wv, Wo, gamma, beta):
    nc = _get_nc()
    in_maps = make_in_maps(x, Wq, Wk, Wv, Wo, gamma, beta)
    res = run_bass_kernel_spmd(nc, in_maps, list(range(NCORES)))
    return assemble(res.results, spatial=tuple(np.asarray(x).shape[2:]))
